# revision 19
# baseline (speedup 1.0000x reference)
"""Trainium2 Bass kernel for GQA attention layer (Llama-style, prefill).

out = softmax((rope(x@wq) @ rope(x@wk)^T)*scale + causal) @ (x@wv) @ wo

Sharding: 8 cores = DP(2 batches) x TP(4 head-groups).  Core c = 4*b + g
handles batch b, q-heads [8g..8g+8), kv-heads [2g..2g+2).  Each core
produces a partial [S, D] o-proj contribution (bf16); the host sums the
4 partials per batch in f32 (row-parallel wo "all-reduce").

v2 design (vs v1 baseline at 1.06 ms):
  - everything bf16 on the wire: w streamed once per s-block in bf16,
    x panels bf16 -> DMA 179MB -> ~75MB, FWL fast weight loads.
  - no l-row matmul chains: P-sums accumulated on DVE, one fused
    broadcast-sum MM (all-ones stationary) per (h,qi), then
    reciprocal_approx_fast (custom DVE op, ~5x faster than reciprocal).
  - V computed in natural [s, d] layout directly (x-panel stationary),
    no PE transposes.
  - software-pipelined emission: attention for q-block sb-1 interleaved
    at MM granularity into projection of s-block sb so exp (ACT) hides
    under dense PE work; o-proj pass1 (qtiles 0..11) interleaves with
    the last attention block, pass2 (qtiles 12..15) at the end.
"""

import numpy as np
import ml_dtypes

import concourse.bass as bass
import concourse.tile as tile
from concourse import bacc, mybir
from concourse.bass_utils import run_bass_kernel_spmd

BF16 = mybir.dt.bfloat16
F32 = mybir.dt.float32

B, S, D, H, KVH, HD = 2, 2048, 4096, 32, 8, 128
G = 4                      # TP groups
HPG = H // G               # q heads per core = 8
KVPG = KVH // G            # kv heads per core = 2
NQK = HPG + KVPG           # 10 roped projection heads (q0-7, k0-1)
SCALE = 1.0 / float(np.sqrt(HD))
SB = 512                   # s-block width
NSB = S // SB              # 4
QB = 512                   # attention q-block width (== SB)
DT = D // 128              # 32 contraction tiles
NKT = S // 128             # 16 key tiles
N_CORES = 8

_CACHE: dict = {}
SHUF_MASK = [i ^ 1 for i in range(32)]   # partition pair-swap (involution)


def _build():
    nc = bacc.Bacc("TRN2", target_bir_lowering=False, debug=False,
                   num_devices=N_CORES)

    # ---- DRAM I/O (all bf16 except none) ----
    x_t = nc.dram_tensor("x_t", [128, NSB, DT, SB], BF16, kind="ExternalInput").ap()
    w_t = nc.dram_tensor("w_t", [128, NQK, DT, 128], BF16, kind="ExternalInput").ap()
    wv_t = nc.dram_tensor("wv_t", [128, DT, 256], BF16, kind="ExternalInput").ap()
    wo_t = nc.dram_tensor("wo_t", [128, 8, HPG, 512], BF16, kind="ExternalInput").ap()
    cosT = nc.dram_tensor("cosT", [128, S], BF16, kind="ExternalInput").ap()
    sinT = nc.dram_tensor("sinT", [128, S], BF16, kind="ExternalInput").ap()
    maskT = nc.dram_tensor("maskT", [128, 128], BF16, kind="ExternalInput").ap()
    ones_sq = nc.dram_tensor("ones_sq", [128, 128], BF16, kind="ExternalInput").ap()
    out = nc.dram_tensor("out", [S, D], BF16, kind="ExternalOutput").ap()

    with tile.TileContext(nc) as tc:
        with (
            tc.tile_pool(name="pers", bufs=1) as pers,
            tc.tile_pool(name="work", bufs=1) as wk,
            tc.tile_pool(name="psum", bufs=1, space="PSUM") as psum,
        ):
            # ---- persistent SBUF ----
            cos_sb = pers.tile([128, S], BF16, tag="cos")
            sin_sb = pers.tile([128, S], BF16, tag="sin")
            mask_sb = pers.tile([128, 128], BF16, tag="mask")
            ones_sb = pers.tile([128, 128], BF16, tag="ones")
            kt_sb = pers.tile([128, KVPG, S], BF16, tag="kt")     # K^T roped
            v_sb = pers.tile([128, NKT, 256], BF16, tag="v")      # V natural
            attnT = pers.tile([128, HPG, S], BF16, tag="attnT")

            # ---------------- emission helpers (generators) ----------------
            def interleave(groups):
                """groups: list of (gen, ratio).  Round-robin until all done,
                pulling `ratio` steps of each per round."""
                groups = [(g, r) for g, r in groups if g is not None]
                alive = [True] * len(groups)
                while any(alive):
                    for i, (g, r) in enumerate(groups):
                        if not alive[i]:
                            continue
                        for _ in range(r):
                            try:
                                next(g)
                            except StopIteration:
                                alive[i] = False
                                break

            def proj_head(sb, h, xpans, qt_cur, wh):
                """Project roped head h (0-7 q, 8-9 k) for s-block sb."""
                acc = psum.tile([128, SB], F32, tag="acc", bufs=3, name="acc")
                for dt_i in range(DT):
                    nc.tensor.matmul(
                        acc, wh[:, dt_i, :], xpans[dt_i // 8][:, dt_i % 8, :],
                        start=(dt_i == 0), stop=(dt_i == DT - 1))
                    yield
                # rope: dst = raw*cos + pairswap(raw)*sin_alt, with the
                # rotate-half sign folded into the sin table.  stream_shuffle
                # (DVE partition pair-swap) replaces the PE perm-matmul.
                scols = slice(sb * SB, (sb + 1) * SB)
                raw = wk.tile([128, SB], BF16, tag="raw", bufs=1, name="raw")
                nc.vector.tensor_copy(raw, acc)
                shuf = wk.tile([128, SB], BF16, tag="shuf", bufs=1,
                               name="shuf")
                nc.vector.stream_shuffle(shuf, raw, SHUF_MASK)
                if h < HPG:
                    dst = qt_cur[:, h, :]
                else:
                    dst = kt_sb[:, h - HPG, scols]
                t1 = wk.tile([128, SB], BF16, tag="t1", bufs=1, name="t1")
                nc.vector.tensor_mul(t1, raw, cos_sb[:, scols])
                t2 = wk.tile([128, SB], BF16, tag="t2", bufs=1, name="t2")
                nc.vector.tensor_mul(t2, shuf, sin_sb[:, scols])
                nc.vector.tensor_add(dst, t1, t2)
                yield

            def proj_v(sb, xpans, wv_sb):
                """V natural: per s-tile, V[s,:256] = sum_dt xpanT @ wv."""
                for st in range(SB // 128):
                    vp = psum.tile([128, 256], F32, tag="mm", bufs=2, name="vp")
                    ssl = slice(st * 128, (st + 1) * 128)
                    for dt_i in range(DT):
                        nc.tensor.matmul(
                            vp, xpans[dt_i // 8][:, dt_i % 8, ssl],
                            wv_sb[:, dt_i, :],
                            start=(dt_i == 0), stop=(dt_i == DT - 1))
                        yield
                    st_g = sb * (SB // 128) + st
                    nc.vector.tensor_copy(v_sb[:, st_g, :], vp)
                    yield

            def attn_scores(h, qi, qt_blk, pt):
                """Scores + exp + causal mask for pair (h, qi).  Diagonal
                k-tile m only needs q-columns [m*128, QB) -- the rest is
                fully masked, so skip computing it (triangular trim)."""
                kvs = h // (HPG // KVPG)
                nkt = 4 * qi + 4
                for kti in range(nkt):
                    m = kti - 4 * qi
                    off = max(0, m) * 128
                    sq = psum.tile([128, QB], F32, tag="sq", bufs=3, name="sq")
                    nc.tensor.matmul(
                        sq[:, :QB - off],
                        kt_sb[:, kvs, kti * 128:(kti + 1) * 128],
                        qt_blk[:, h, off:], start=True, stop=True)
                    nc.scalar.activation(
                        pt[:, kti, off:], sq[:, :QB - off],
                        mybir.ActivationFunctionType.Exp, scale=SCALE)
                    if m >= 0:
                        nc.vector.tensor_mul(
                            pt[:, kti, off:off + 128],
                            pt[:, kti, off:off + 128], mask_sb)
                    yield

            def attn_av(h, qi, pt):
                """AV chain + softmax denom + normalize for pair (h, qi)."""
                kvs = h // (HPG // KVPG)
                nkt = 4 * qi + 4
                qcols = slice(qi * QB, (qi + 1) * QB)
                oT = psum.tile([128, QB], F32, tag="acc", bufs=3, name="oT")
                pts = wk.tile([128, QB], BF16, tag="pts", bufs=1, name="pts")
                for kti in range(nkt):
                    m = kti - 4 * qi
                    off = max(0, m) * 128
                    nc.tensor.matmul(
                        oT[:, off:], v_sb[:, kti, kvs * 128:(kvs + 1) * 128],
                        pt[:, kti, off:],
                        start=(kti == 0), stop=(kti == nkt - 1))
                    if kti == 0:
                        nc.vector.tensor_copy(pts, pt[:, 0, :])
                    else:
                        nc.vector.tensor_add(pts[:, off:], pts[:, off:],
                                             pt[:, kti, off:])
                    yield
                bc = psum.tile([128, QB], F32, tag="mm", bufs=2, name="bc")
                nc.tensor.matmul(bc, ones_sb, pts, start=True, stop=True)
                rinv = wk.tile([128, QB], F32, tag="rinv", bufs=1, name="rinv")
                nc.vector.reciprocal_approx_fast(out=rinv, in_=bc)
                nc.vector.tensor_mul(attnT[:, h, qcols], oT, rinv)
                yield

            def st3_block(dblk, qtiles, wo_sb):
                """o-proj: out[qsl, dblk] = sum_hp attnT[:,hp,qsl].T @ wo."""
                for qt_i in qtiles:
                    qsl = slice(qt_i * 128, (qt_i + 1) * 128)
                    ops = psum.tile([128, 512], F32, tag="acc", bufs=3,
                                    name="ops")
                    for hp in range(HPG):
                        nc.tensor.matmul(
                            ops, attnT[:, hp, qsl], wo_sb[:, hp, :],
                            start=(hp == 0), stop=(hp == HPG - 1))
                        yield
                    o_sb = wk.tile([128, 512], BF16, tag="osb", bufs=2,
                                   name="o_sb")
                    nc.vector.tensor_copy(o_sb, ops)
                    nc.sync.dma_start(
                        out=out[qsl, dblk * 512:(dblk + 1) * 512], in_=o_sb)
                    yield

            # ---------------- stage 1+2: projection + attention ----------------
            with tc.tile_pool(name="st1", bufs=1) as st1:
                def load_xpans(sb):
                    # sb0 on the sync queue (immediate); later blocks on the
                    # gpsimd queue, whose FIFO is tile-gated by wh loads --
                    # keeps the startup DMA burst small so the critical
                    # first-chain transfers get the bandwidth.
                    eng = nc.sync if sb == 0 else nc.gpsimd
                    pans = []
                    for hp in range(4):
                        xp = st1.tile([128, 8, SB], BF16, tag="xpan", bufs=6,
                                      name="xp")
                        eng.dma_start(
                            out=xp, in_=x_t[:, sb, hp * 8:(hp + 1) * 8, :])
                        pans.append(xp)
                    return pans

                def load_wh(h, eng=None):
                    wh = st1.tile([128, DT, 128], BF16, tag="wh", bufs=3,
                                  name="wh")
                    (eng or nc.gpsimd).dma_start(out=wh, in_=w_t[:, h])
                    return wh

                # critical-path DMAs first: k0/k1 weights (gpsimd queue) and
                # the sb0 x panels (sync queue); the persistent tables and wv
                # are not needed until ~20us in and must not sit ahead of
                # them in the queue-FIFO at saturated HBM bandwidth.  The k0
                # weights are split in 4 chunks so the first chain can start
                # on chunk 0 while the rest stream.
                # need-ordered critical stream, all on the sync queue:
                # descriptors on one queue complete in issue order at full
                # bandwidth, so interleave k0-weight chunks with the x
                # quarters exactly in first-chain consumption order.
                wh_k0 = st1.tile([128, DT, 128], BF16, tag="wh", bufs=3,
                                 name="wh_k0")
                xpans0 = []
                for ck in range(4):
                    nc.sync.dma_start(out=wh_k0[:, ck * 8:(ck + 1) * 8, :],
                                      in_=w_t[:, HPG, ck * 8:(ck + 1) * 8, :])
                    xp = st1.tile([128, 8, SB], BF16, tag="xpan", bufs=6,
                                  name="xp")
                    nc.sync.dma_start(out=xp, in_=x_t[:, 0, ck * 8:(ck + 1) * 8, :])
                    xpans0.append(xp)
                xpans = {0: xpans0}
                wh_k1 = load_wh(HPG + 1, eng=nc.sync)
                # PE warm-up: ~60 junk matmuls release the HAM clock gate
                # (4/8 -> 8/8) while the first DMAs land.
                scr = st1.tile([128, 64], BF16, tag="scr", name="scr")
                nc.vector.memset(scr, 0.0)
                wup = psum.tile([128, 64], F32, tag="mm", bufs=2, name="wup")
                for _ in range(60):
                    nc.tensor.matmul(wup[:64, :], scr, scr, start=True,
                                     stop=True, skip_group_check=True)
                nc.sync.dma_start(out=cos_sb, in_=cosT)
                nc.sync.dma_start(out=sin_sb, in_=sinT)
                wh_q0 = load_wh(0, eng=nc.sync)
                nc.gpsimd.dma_start(out=mask_sb, in_=maskT)
                nc.gpsimd.dma_start(out=ones_sb, in_=ones_sq)
                wv_sb = st1.tile([128, DT, 256], BF16, tag="wv")

                def junk_mms(n):
                    for _ in range(n):
                        nc.tensor.matmul(wup[:64, :], scr, scr, start=True,
                                         stop=True, skip_group_check=True)
                        yield

                qt_blocks = {}

                for sb in range(NSB):
                    qt_cur = wk.tile([128, HPG, SB], BF16, tag="qt", bufs=2,
                                     name="qt_cur")
                    qt_blocks[sb] = qt_cur
                    # slots 0..11: [k0] [k1] [v] [q0] .. [q7] [tail] ; attn of
                    # q-block sb-1: scores j in slot 3+j, av j in slot 4+j
                    # slot -> work: 0:k0 1:k1 2:q0..7:q5 8:v 9:q6 10:q7 11:-
                    # (v late so its wv table has ~75us of DMA slack)
                    qslot = lambda j: 2 + j if j < 6 else 3 + j
                    slots = [[] for _ in range(12)]
                    if sb == 0:
                        whk0, whk1 = wh_k0, wh_k1
                    else:
                        whk0, whk1 = load_wh(HPG), load_wh(HPG + 1)
                    slots[0].append((proj_head(sb, HPG, xpans[sb], qt_cur,
                                               whk0), 2))
                    slots[1].append((proj_head(sb, HPG + 1, xpans[sb], qt_cur,
                                               whk1), 2))
                    if sb == 0:
                        # keep HAM warm while the first chains are DMA-paced
                        slots[0].append((junk_mms(16), 1))
                        slots[1].append((junk_mms(8), 1))
                    slots[8].append((proj_v(sb, xpans[sb], wv_sb), 2))
                    for j in range(HPG):
                        whj = wh_q0 if (sb == 0 and j == 0) else load_wh(j)
                        slots[qslot(j)].append(
                            (proj_head(sb, j, xpans[sb], qt_cur, whj), 2))
                        if sb == 0 and j == 1:
                            nc.gpsimd.dma_start(out=wv_sb, in_=wv_t)
                    if sb + 1 < NSB:
                        xpans[sb + 1] = load_xpans(sb + 1)
                    if sb > 0:
                        qi = sb - 1
                        qt_prev = qt_blocks[sb - 1]
                        pt_tiles = {}
                        for j in range(HPG):
                            pt = wk.tile([128, 4 * qi + 4, QB], BF16, tag="pt",
                                         bufs=2, padded_shape=[128, NKT, QB],
                                         name="pt")
                            pt_tiles[j] = pt
                            slots[qslot(j)].append(
                                (attn_scores(j, qi, qt_prev, pt), 1))
                            slots[qslot(j) + 1].append((attn_av(j, qi, pt), 1))
                    for group in slots:
                        interleave(group)

            # ---------------- tail: qi=3 attention + o-proj ----------------
            with tc.tile_pool(name="st3", bufs=1) as st3:
                # wo on the gpsimd queue: the sync FIFO blocks on out-DMA
                # waits, which would delay the next wo issue to block end
                def load_wo(dblk):
                    wo_sb = st3.tile([128, HPG, 512], BF16, tag="wo", bufs=6,
                                     name="wo_sb")
                    nc.gpsimd.dma_start(out=wo_sb, in_=wo_t[:, dblk])
                    return wo_sb

                qi = 3
                qt_prev = qt_blocks[3]
                pt_list = []
                for j in range(HPG):
                    pt = wk.tile([128, NKT, QB], BF16, tag="pt", bufs=2,
                                 name="pt")
                    pt_list.append(pt)
                    group = [(attn_scores(j, qi, qt_prev, pt), 1)]
                    if j > 0:
                        group.append((attn_av(j - 1, qi, pt_list[j - 1]), 1))
                    group.append((st3_block(j, range(12), load_wo(j)), 3))
                    interleave(group)
                # last av fully emitted BEFORE pass-2 o-proj (qtiles 12..15
                # read attnT[:,7,...] -- emitting them earlier would wedge
                # the in-order PE queue behind av(7)'s own matmuls)
                interleave([(attn_av(HPG - 1, qi, pt_list[HPG - 1]), 1)])
                for dblk in range(8):
                    interleave([(st3_block(dblk, range(12, 16),
                                           load_wo(dblk)), 1)])

    nc.compile()
    return nc


def _host_inputs(x, wq, wk_, wv, wo, cos, sin):
    """Build the 8 per-core input maps (all host-side prep, bf16)."""
    bf = ml_dtypes.bfloat16
    x = np.asarray(x, np.float32)
    wq = np.asarray(wq, np.float32)
    wk_ = np.asarray(wk_, np.float32)
    wv = np.asarray(wv, np.float32)
    wo = np.asarray(wo, np.float32)
    cos = np.asarray(cos, np.float32)
    sin = np.asarray(sin, np.float32)

    cosT = np.repeat(cos.T, 2, axis=0).astype(bf)   # [128, S]
    sinT = np.repeat(sin.T, 2, axis=0)
    sinT[0::2, :] = -sinT[0::2, :]   # rotate-half sign for even lanes
    sinT = sinT.astype(bf)
    kk = np.arange(128)[:, None]
    qq = np.arange(128)[None, :]
    maskT = (qq >= kk).astype(bf)   # within-tile causal triangle
    ones_sq = np.ones((128, 128), bf)

    def tile_w(w_col):  # [D, 128] -> [128, DT, 128]
        return w_col.reshape(DT, 128, 128).transpose(1, 0, 2)

    x_ts = []
    for b in range(B):
        xT = np.ascontiguousarray(x[b].T)                        # [D, S]
        xt = xT.reshape(DT, 128, NSB, SB).transpose(1, 2, 0, 3)  # [128,NSB,DT,SB]
        x_ts.append(np.ascontiguousarray(xt).astype(bf))

    in_maps = []
    for core in range(N_CORES):
        b, g = divmod(core, G)
        w_t = np.empty((128, NQK, DT, 128), np.float32)
        for j in range(HPG):
            w_t[:, j] = tile_w(wq[:, (g * HPG + j) * 128:(g * HPG + j + 1) * 128])
        for j in range(KVPG):
            w_t[:, HPG + j] = tile_w(
                wk_[:, (g * KVPG + j) * 128:(g * KVPG + j + 1) * 128])
        wv_g = wv[:, g * KVPG * 128:(g + 1) * KVPG * 128]        # [D, 256]
        wv_t = wv_g.reshape(DT, 128, 256).transpose(1, 0, 2)     # [128, DT, 256]
        wo_g = wo[g * HPG * HD:(g + 1) * HPG * HD, :]            # [1024, D]
        wo_t = np.ascontiguousarray(
            wo_g.reshape(HPG, 128, 8, 512).transpose(1, 2, 0, 3)
        ).astype(bf)                                             # [128,8,HPG,512]
        in_maps.append({
            "x_t": x_ts[b],
            "w_t": np.ascontiguousarray(w_t).astype(bf),
            "wv_t": np.ascontiguousarray(wv_t).astype(bf),
            "wo_t": wo_t,
            "cosT": cosT, "sinT": sinT, "maskT": maskT,
            "ones_sq": ones_sq,
        })
    return in_maps


def kernel(x, wq, wk, wv, wo, cos, sin, mask, start_pos):
    assert int(start_pos) == 0, "kernel compiled for prefill (start_pos=0)"
    if "nc" not in _CACHE:
        _CACHE["nc"] = _build()
    nc = _CACHE["nc"]
    in_maps = _host_inputs(x, wq, wk, wv, wo, cos, sin)
    res = run_bass_kernel_spmd(nc, in_maps, list(range(N_CORES)))
    outs = [np.asarray(res.results[c]["out"], np.float32)
            for c in range(N_CORES)]
    full = np.empty((B, S, D), np.float32)
    for b in range(B):
        full[b] = outs[4 * b + 0] + outs[4 * b + 1] + outs[4 * b + 2] + outs[4 * b + 3]
    return full


# revision 20
# speedup vs baseline: 1.1923x; 1.1923x over previous
"""Trainium2 Bass kernel for GQA attention layer (Llama-style, prefill).

out = softmax((rope(x@wq) @ rope(x@wk)^T)*scale + causal) @ (x@wv) @ wo

Sharding: 8 cores = DP(2 batches) x TP(4 head-groups).  Core c = 4*b + g
handles batch b, q-heads [8g..8g+8), kv-heads [2g..2g+2).  Each core
produces a partial [S, D] o-proj contribution (bf16); the host sums the
4 partials per batch in f32 (row-parallel wo "all-reduce").

v2 design (vs v1 baseline at 1.06 ms):
  - everything bf16 on the wire: w streamed once per s-block in bf16,
    x panels bf16 -> DMA 179MB -> ~75MB, FWL fast weight loads.
  - no l-row matmul chains: P-sums accumulated on DVE, one fused
    broadcast-sum MM (all-ones stationary) per (h,qi), then
    reciprocal_approx_fast (custom DVE op, ~5x faster than reciprocal).
  - V computed in natural [s, d] layout directly (x-panel stationary),
    no PE transposes.
  - software-pipelined emission: attention for q-block sb-1 interleaved
    at MM granularity into projection of s-block sb so exp (ACT) hides
    under dense PE work; o-proj pass1 (qtiles 0..11) interleaves with
    the last attention block, pass2 (qtiles 12..15) at the end.
"""

import numpy as np
import ml_dtypes

import concourse.bass as bass
import concourse.tile as tile
from concourse import bacc, mybir
from concourse.bass_utils import run_bass_kernel_spmd

BF16 = mybir.dt.bfloat16
F32 = mybir.dt.float32

B, S, D, H, KVH, HD = 2, 2048, 4096, 32, 8, 128
G = 4                      # TP groups
HPG = H // G               # q heads per core = 8
KVPG = KVH // G            # kv heads per core = 2
NQK = HPG + KVPG           # 10 roped projection heads (q0-7, k0-1)
SCALE = 1.0 / float(np.sqrt(HD))
SB = 512                   # s-block width
NSB = S // SB              # 4
QB = 512                   # attention q-block width (== SB)
DT = D // 128              # 32 contraction tiles
NKT = S // 128             # 16 key tiles
N_CORES = 8

_CACHE: dict = {}
SHUF_MASK = [i ^ 1 for i in range(32)]   # partition pair-swap (involution)


def _build():
    nc = bacc.Bacc("TRN2", target_bir_lowering=False, debug=False,
                   num_devices=N_CORES)

    # ---- DRAM I/O (all bf16 except none) ----
    x_t = nc.dram_tensor("x_t", [128, NSB, DT, SB], BF16, kind="ExternalInput").ap()
    w_t = nc.dram_tensor("w_t", [128, NQK, DT, 128], BF16, kind="ExternalInput").ap()
    wv_t = nc.dram_tensor("wv_t", [128, DT, 256], BF16, kind="ExternalInput").ap()
    wo_t = nc.dram_tensor("wo_t", [128, 8, HPG, 512], BF16, kind="ExternalInput").ap()
    cosT = nc.dram_tensor("cosT", [128, S], BF16, kind="ExternalInput").ap()
    sinT = nc.dram_tensor("sinT", [128, S], BF16, kind="ExternalInput").ap()
    maskT = nc.dram_tensor("maskT", [128, 128], BF16, kind="ExternalInput").ap()
    ones_sq = nc.dram_tensor("ones_sq", [128, 128], BF16, kind="ExternalInput").ap()
    out = nc.dram_tensor("out", [S, D], BF16, kind="ExternalOutput").ap()

    with tile.TileContext(nc) as tc:
        with (
            tc.tile_pool(name="pers", bufs=1) as pers,
            tc.tile_pool(name="work", bufs=1) as wk,
            tc.tile_pool(name="psum", bufs=1, space="PSUM") as psum,
        ):
            # ---- persistent SBUF ----
            cos_sb = pers.tile([128, S], BF16, tag="cos")
            sin_sb = pers.tile([128, S], BF16, tag="sin")
            mask_sb = pers.tile([128, 128], BF16, tag="mask")
            ones_sb = pers.tile([128, 128], BF16, tag="ones")
            kt_sb = pers.tile([128, KVPG, S], BF16, tag="kt")     # K^T roped
            v_sb = pers.tile([128, NKT, 256], BF16, tag="v")      # V natural
            attnT = pers.tile([128, HPG, S], BF16, tag="attnT")

            # ---------------- emission helpers (generators) ----------------
            def interleave(groups):
                """groups: list of (gen, ratio).  Round-robin until all done,
                pulling `ratio` steps of each per round."""
                groups = [(g, r) for g, r in groups if g is not None]
                alive = [True] * len(groups)
                while any(alive):
                    for i, (g, r) in enumerate(groups):
                        if not alive[i]:
                            continue
                        for _ in range(r):
                            try:
                                next(g)
                            except StopIteration:
                                alive[i] = False
                                break

            def proj_head(sb, h, xpans, qt_cur, wh):
                """Project roped head h (0-7 q, 8-9 k) for s-block sb."""
                acc = psum.tile([128, SB], F32, tag="acc", bufs=2, name="acc")
                for dt_i in range(DT):
                    nc.tensor.matmul(
                        acc, wh[:, dt_i, :], xpans[dt_i // 8][:, dt_i % 8, :],
                        start=(dt_i == 0), stop=(dt_i == DT - 1))
                    yield
                # rope: dst = raw*cos + pairswap(raw)*sin_alt, with the
                # rotate-half sign folded into the sin table.  stream_shuffle
                # (DVE partition pair-swap) replaces the PE perm-matmul.
                scols = slice(sb * SB, (sb + 1) * SB)
                raw = wk.tile([128, SB], BF16, tag="raw", bufs=1, name="raw")
                nc.vector.tensor_copy(raw, acc)
                shuf = wk.tile([128, SB], BF16, tag="shuf", bufs=1,
                               name="shuf")
                nc.vector.stream_shuffle(shuf, raw, SHUF_MASK)
                if h < HPG:
                    dst = qt_cur[:, h, :]
                else:
                    dst = kt_sb[:, h - HPG, scols]
                t1 = wk.tile([128, SB], BF16, tag="t1", bufs=1, name="t1")
                nc.vector.tensor_mul(t1, raw, cos_sb[:, scols])
                t2 = wk.tile([128, SB], BF16, tag="t2", bufs=1, name="t2")
                nc.vector.tensor_mul(t2, shuf, sin_sb[:, scols])
                nc.vector.tensor_add(dst, t1, t2)
                yield

            def proj_v(sb, xpans, wv_sb):
                """V natural: per s-tile, V[s,:256] = sum_dt xpanT @ wv."""
                for st in range(SB // 128):
                    vp = psum.tile([128, 256], F32, tag="mm", bufs=2, name="vp")
                    ssl = slice(st * 128, (st + 1) * 128)
                    for dt_i in range(DT):
                        nc.tensor.matmul(
                            vp, xpans[dt_i // 8][:, dt_i % 8, ssl],
                            wv_sb[:, dt_i, :],
                            start=(dt_i == 0), stop=(dt_i == DT - 1))
                        yield
                    st_g = sb * (SB // 128) + st
                    nc.vector.tensor_copy(v_sb[:, st_g, :], vp)
                    yield

            def attn_scores(h, qi, qt_blk, pt):
                """Scores + exp + causal mask for pair (h, qi).  Diagonal
                k-tile m only needs q-columns [m*128, QB) -- the rest is
                fully masked, so skip computing it (triangular trim)."""
                kvs = h // (HPG // KVPG)
                nkt = 4 * qi + 4
                for kti in range(nkt):
                    m = kti - 4 * qi
                    off = max(0, m) * 128
                    sq = psum.tile([128, QB], F32, tag="sq", bufs=2, name="sq")
                    nc.tensor.matmul(
                        sq[:, :QB - off],
                        kt_sb[:, kvs, kti * 128:(kti + 1) * 128],
                        qt_blk[:, h, off:], start=True, stop=True)
                    nc.scalar.activation(
                        pt[:, kti, off:], sq[:, :QB - off],
                        mybir.ActivationFunctionType.Exp, scale=SCALE)
                    if m >= 0:
                        nc.vector.tensor_mul(
                            pt[:, kti, off:off + 128],
                            pt[:, kti, off:off + 128], mask_sb)
                    yield

            def attn_av(h, qi, pt):
                """AV chain + softmax denom + normalize for pair (h, qi)."""
                kvs = h // (HPG // KVPG)
                nkt = 4 * qi + 4
                qcols = slice(qi * QB, (qi + 1) * QB)
                oT = psum.tile([128, QB], F32, tag="oT", bufs=2, name="oT")
                pts = wk.tile([128, QB], BF16, tag="pts", bufs=1, name="pts")
                for kti in range(nkt):
                    m = kti - 4 * qi
                    off = max(0, m) * 128
                    nc.tensor.matmul(
                        oT[:, off:], v_sb[:, kti, kvs * 128:(kvs + 1) * 128],
                        pt[:, kti, off:],
                        start=(kti == 0), stop=(kti == nkt - 1))
                    if kti == 0:
                        nc.vector.tensor_copy(pts, pt[:, 0, :])
                    else:
                        nc.vector.tensor_add(pts[:, off:], pts[:, off:],
                                             pt[:, kti, off:])
                    yield
                bc = psum.tile([128, QB], F32, tag="mm", bufs=2, name="bc")
                nc.tensor.matmul(bc, ones_sb, pts, start=True, stop=True)
                rinv = wk.tile([128, QB], F32, tag="rinv", bufs=1, name="rinv")
                nc.vector.reciprocal_approx_fast(out=rinv, in_=bc)
                nc.vector.tensor_mul(attnT[:, h, qcols], oT, rinv)
                yield

            def st3_block(dblk, qtiles, wo_sb):
                """o-proj: out[qsl, dblk] = sum_hp attnT[:,hp,qsl].T @ wo."""
                for qt_i in qtiles:
                    qsl = slice(qt_i * 128, (qt_i + 1) * 128)
                    ops = psum.tile([128, 512], F32, tag="acc", bufs=2,
                                    name="ops")
                    for hp in range(HPG):
                        nc.tensor.matmul(
                            ops, attnT[:, hp, qsl], wo_sb[:, hp, :],
                            start=(hp == 0), stop=(hp == HPG - 1))
                        yield
                    o_sb = wk.tile([128, 512], BF16, tag="osb", bufs=2,
                                   name="o_sb")
                    nc.vector.tensor_copy(o_sb, ops)
                    nc.sync.dma_start(
                        out=out[qsl, dblk * 512:(dblk + 1) * 512], in_=o_sb)
                    yield

            # ---------------- stage 1+2: projection + attention ----------------
            with tc.tile_pool(name="st1", bufs=1) as st1:
                def load_xpans(sb):
                    # sb0 on the sync queue (immediate); later blocks on the
                    # gpsimd queue, whose FIFO is tile-gated by wh loads --
                    # keeps the startup DMA burst small so the critical
                    # first-chain transfers get the bandwidth.
                    eng = nc.sync if sb == 0 else nc.gpsimd
                    pans = []
                    for hp in range(4):
                        xp = st1.tile([128, 8, SB], BF16, tag="xpan", bufs=6,
                                      name="xp")
                        eng.dma_start(
                            out=xp, in_=x_t[:, sb, hp * 8:(hp + 1) * 8, :])
                        pans.append(xp)
                    return pans

                def load_wh(h, eng=None):
                    wh = st1.tile([128, DT, 128], BF16, tag="wh", bufs=3,
                                  name="wh")
                    (eng or nc.gpsimd).dma_start(out=wh, in_=w_t[:, h])
                    return wh

                # critical-path DMAs first: k0/k1 weights (gpsimd queue) and
                # the sb0 x panels (sync queue); the persistent tables and wv
                # are not needed until ~20us in and must not sit ahead of
                # them in the queue-FIFO at saturated HBM bandwidth.  The k0
                # weights are split in 4 chunks so the first chain can start
                # on chunk 0 while the rest stream.
                # need-ordered critical stream, all on the sync queue:
                # descriptors on one queue complete in issue order at full
                # bandwidth, so interleave k0-weight chunks with the x
                # quarters exactly in first-chain consumption order.
                wh_k0 = st1.tile([128, DT, 128], BF16, tag="wh", bufs=3,
                                 name="wh_k0")
                xpans0 = []
                for ck in range(4):
                    eng_w = nc.sync if ck % 2 == 0 else nc.gpsimd
                    eng_x = nc.gpsimd if ck % 2 == 0 else nc.sync
                    eng_w.dma_start(out=wh_k0[:, ck * 8:(ck + 1) * 8, :],
                                    in_=w_t[:, HPG, ck * 8:(ck + 1) * 8, :])
                    xp = st1.tile([128, 8, SB], BF16, tag="xpan", bufs=6,
                                  name="xp")
                    eng_x.dma_start(out=xp, in_=x_t[:, 0, ck * 8:(ck + 1) * 8, :])
                    xpans0.append(xp)
                xpans = {0: xpans0}
                wh_k1 = load_wh(HPG + 1, eng=nc.sync)
                # PE warm-up: ~60 junk matmuls release the HAM clock gate
                # (4/8 -> 8/8) while the first DMAs land.
                scr = st1.tile([128, 64], BF16, tag="scr", name="scr")
                nc.vector.memset(scr, 0.0)
                wup = psum.tile([128, 64], F32, tag="mm", bufs=2, name="wup")
                for _ in range(60):
                    nc.tensor.matmul(wup[:64, :], scr, scr, start=True,
                                     stop=True, skip_group_check=True)
                nc.gpsimd.dma_start(out=cos_sb, in_=cosT)
                nc.gpsimd.dma_start(out=sin_sb, in_=sinT)
                wh_q0 = load_wh(0, eng=nc.sync)
                nc.gpsimd.dma_start(out=mask_sb, in_=maskT)
                nc.gpsimd.dma_start(out=ones_sb, in_=ones_sq)
                wv_sb = st1.tile([128, DT, 256], BF16, tag="wv")

                def junk_mms(n):
                    for _ in range(n):
                        nc.tensor.matmul(wup[:64, :], scr, scr, start=True,
                                         stop=True, skip_group_check=True)
                        yield

                qt_blocks = {}

                for sb in range(NSB):
                    qt_cur = wk.tile([128, HPG, SB], BF16, tag="qt", bufs=2,
                                     name="qt_cur")
                    qt_blocks[sb] = qt_cur
                    # slots 0..11: [k0] [k1] [v] [q0] .. [q7] [tail] ; attn of
                    # q-block sb-1: scores j in slot 3+j, av j in slot 4+j
                    # slot -> work: 0:k0 1:k1 2:q0..7:q5 8:v 9:q6 10:q7 11:-
                    # (v late so its wv table has ~75us of DMA slack)
                    qslot = lambda j: 2 + j if j < 6 else 3 + j
                    slots = [[] for _ in range(12)]
                    if sb == 0:
                        whk0, whk1 = wh_k0, wh_k1
                    else:
                        whk0, whk1 = load_wh(HPG), load_wh(HPG + 1)
                    slots[0].append((proj_head(sb, HPG, xpans[sb], qt_cur,
                                               whk0), 2))
                    slots[1].append((proj_head(sb, HPG + 1, xpans[sb], qt_cur,
                                               whk1), 2))
                    if sb == 0:
                        # keep HAM warm while the first chains are DMA-paced
                        slots[0].append((junk_mms(16), 1))
                        slots[1].append((junk_mms(8), 1))
                    slots[8].append((proj_v(sb, xpans[sb], wv_sb), 2))
                    for j in range(HPG):
                        whj = wh_q0 if (sb == 0 and j == 0) else load_wh(j)
                        slots[qslot(j)].append(
                            (proj_head(sb, j, xpans[sb], qt_cur, whj), 2))
                        if sb == 0 and j == 1:
                            nc.gpsimd.dma_start(out=wv_sb, in_=wv_t)
                    if sb + 1 < NSB:
                        xpans[sb + 1] = load_xpans(sb + 1)
                    if sb > 0:
                        qi = sb - 1
                        qt_prev = qt_blocks[sb - 1]
                        pt_tiles = {}
                        for j in range(HPG):
                            pt = wk.tile([128, 4 * qi + 4, QB], BF16, tag="pt",
                                         bufs=2, padded_shape=[128, NKT, QB],
                                         name="pt")
                            pt_tiles[j] = pt
                            slots[qslot(j)].append(
                                (attn_scores(j, qi, qt_prev, pt), 1))
                            slots[qslot(j) + 1].append((attn_av(j, qi, pt), 1))
                    for group in slots:
                        interleave(group)

            # ---------------- tail: qi=3 attention + o-proj ----------------
            with tc.tile_pool(name="st3", bufs=1) as st3:
                # wo on the gpsimd queue: the sync FIFO blocks on out-DMA
                # waits, which would delay the next wo issue to block end
                def load_wo(dblk):
                    wo_sb = st3.tile([128, HPG, 512], BF16, tag="wo", bufs=6,
                                     name="wo_sb")
                    nc.gpsimd.dma_start(out=wo_sb, in_=wo_t[:, dblk])
                    return wo_sb

                qi = 3
                qt_prev = qt_blocks[3]
                pt_list = []
                for j in range(HPG):
                    pt = wk.tile([128, NKT, QB], BF16, tag="pt", bufs=2,
                                 name="pt")
                    pt_list.append(pt)
                    group = [(attn_scores(j, qi, qt_prev, pt), 1)]
                    if j > 0:
                        group.append((attn_av(j - 1, qi, pt_list[j - 1]), 1))
                    group.append((st3_block(j, range(12), load_wo(j)), 3))
                    interleave(group)
                # last av fully emitted BEFORE pass-2 o-proj (qtiles 12..15
                # read attnT[:,7,...] -- emitting them earlier would wedge
                # the in-order PE queue behind av(7)'s own matmuls)
                interleave([(attn_av(HPG - 1, qi, pt_list[HPG - 1]), 1)])
                for dblk in range(8):
                    interleave([(st3_block(dblk, range(12, 16),
                                           load_wo(dblk)), 1)])

    nc.compile()
    return nc


def _host_inputs(x, wq, wk_, wv, wo, cos, sin):
    """Build the 8 per-core input maps (all host-side prep, bf16)."""
    bf = ml_dtypes.bfloat16
    x = np.asarray(x, np.float32)
    wq = np.asarray(wq, np.float32)
    wk_ = np.asarray(wk_, np.float32)
    wv = np.asarray(wv, np.float32)
    wo = np.asarray(wo, np.float32)
    cos = np.asarray(cos, np.float32)
    sin = np.asarray(sin, np.float32)

    cosT = np.repeat(cos.T, 2, axis=0).astype(bf)   # [128, S]
    sinT = np.repeat(sin.T, 2, axis=0)
    sinT[0::2, :] = -sinT[0::2, :]   # rotate-half sign for even lanes
    sinT = sinT.astype(bf)
    kk = np.arange(128)[:, None]
    qq = np.arange(128)[None, :]
    maskT = (qq >= kk).astype(bf)   # within-tile causal triangle
    ones_sq = np.ones((128, 128), bf)

    def tile_w(w_col):  # [D, 128] -> [128, DT, 128]
        return w_col.reshape(DT, 128, 128).transpose(1, 0, 2)

    x_ts = []
    for b in range(B):
        xT = np.ascontiguousarray(x[b].T)                        # [D, S]
        xt = xT.reshape(DT, 128, NSB, SB).transpose(1, 2, 0, 3)  # [128,NSB,DT,SB]
        x_ts.append(np.ascontiguousarray(xt).astype(bf))

    in_maps = []
    for core in range(N_CORES):
        b, g = divmod(core, G)
        w_t = np.empty((128, NQK, DT, 128), np.float32)
        for j in range(HPG):
            w_t[:, j] = tile_w(wq[:, (g * HPG + j) * 128:(g * HPG + j + 1) * 128])
        for j in range(KVPG):
            w_t[:, HPG + j] = tile_w(
                wk_[:, (g * KVPG + j) * 128:(g * KVPG + j + 1) * 128])
        wv_g = wv[:, g * KVPG * 128:(g + 1) * KVPG * 128]        # [D, 256]
        wv_t = wv_g.reshape(DT, 128, 256).transpose(1, 0, 2)     # [128, DT, 256]
        wo_g = wo[g * HPG * HD:(g + 1) * HPG * HD, :]            # [1024, D]
        wo_t = np.ascontiguousarray(
            wo_g.reshape(HPG, 128, 8, 512).transpose(1, 2, 0, 3)
        ).astype(bf)                                             # [128,8,HPG,512]
        in_maps.append({
            "x_t": x_ts[b],
            "w_t": np.ascontiguousarray(w_t).astype(bf),
            "wv_t": np.ascontiguousarray(wv_t).astype(bf),
            "wo_t": wo_t,
            "cosT": cosT, "sinT": sinT, "maskT": maskT,
            "ones_sq": ones_sq,
        })
    return in_maps


def kernel(x, wq, wk, wv, wo, cos, sin, mask, start_pos):
    assert int(start_pos) == 0, "kernel compiled for prefill (start_pos=0)"
    if "nc" not in _CACHE:
        _CACHE["nc"] = _build()
    nc = _CACHE["nc"]
    in_maps = _host_inputs(x, wq, wk, wv, wo, cos, sin)
    res = run_bass_kernel_spmd(nc, in_maps, list(range(N_CORES)))
    outs = [np.asarray(res.results[c]["out"], np.float32)
            for c in range(N_CORES)]
    full = np.empty((B, S, D), np.float32)
    for b in range(B):
        full[b] = outs[4 * b + 0] + outs[4 * b + 1] + outs[4 * b + 2] + outs[4 * b + 3]
    return full


# revision 21
# speedup vs baseline: 1.1939x; 1.0013x over previous
"""Trainium2 Bass kernel for GQA attention layer (Llama-style, prefill).

out = softmax((rope(x@wq) @ rope(x@wk)^T)*scale + causal) @ (x@wv) @ wo

Sharding: 8 cores = DP(2 batches) x TP(4 head-groups).  Core c = 4*b + g
handles batch b, q-heads [8g..8g+8), kv-heads [2g..2g+2).  Each core
produces a partial [S, D] o-proj contribution (bf16); the host sums the
4 partials per batch in f32 (row-parallel wo "all-reduce").

v2 design (vs v1 baseline at 1.06 ms):
  - everything bf16 on the wire: w streamed once per s-block in bf16,
    x panels bf16 -> DMA 179MB -> ~75MB, FWL fast weight loads.
  - no l-row matmul chains: P-sums accumulated on DVE, one fused
    broadcast-sum MM (all-ones stationary) per (h,qi), then
    reciprocal_approx_fast (custom DVE op, ~5x faster than reciprocal).
  - V computed in natural [s, d] layout directly (x-panel stationary),
    no PE transposes.
  - software-pipelined emission: attention for q-block sb-1 interleaved
    at MM granularity into projection of s-block sb so exp (ACT) hides
    under dense PE work; o-proj pass1 (qtiles 0..11) interleaves with
    the last attention block, pass2 (qtiles 12..15) at the end.
"""

import numpy as np
import ml_dtypes

import concourse.bass as bass
import concourse.tile as tile
from concourse import bacc, mybir
from concourse.bass_utils import run_bass_kernel_spmd

BF16 = mybir.dt.bfloat16
F32 = mybir.dt.float32

B, S, D, H, KVH, HD = 2, 2048, 4096, 32, 8, 128
G = 4                      # TP groups
HPG = H // G               # q heads per core = 8
KVPG = KVH // G            # kv heads per core = 2
NQK = HPG + KVPG           # 10 roped projection heads (q0-7, k0-1)
SCALE = 1.0 / float(np.sqrt(HD))
SB = 512                   # s-block width
NSB = S // SB              # 4
QB = 512                   # attention q-block width (== SB)
DT = D // 128              # 32 contraction tiles
NKT = S // 128             # 16 key tiles
N_CORES = 8

_CACHE: dict = {}
SHUF_MASK = [i ^ 1 for i in range(32)]   # partition pair-swap (involution)


def _build():
    nc = bacc.Bacc("TRN2", target_bir_lowering=False, debug=False,
                   num_devices=N_CORES)

    # ---- DRAM I/O (all bf16 except none) ----
    x_t = nc.dram_tensor("x_t", [128, NSB, DT, SB], BF16, kind="ExternalInput").ap()
    w_t = nc.dram_tensor("w_t", [128, NQK, DT, 128], BF16, kind="ExternalInput").ap()
    wv_t = nc.dram_tensor("wv_t", [128, DT, 256], BF16, kind="ExternalInput").ap()
    wo_t = nc.dram_tensor("wo_t", [128, 8, HPG, 512], BF16, kind="ExternalInput").ap()
    cosT = nc.dram_tensor("cosT", [128, S], BF16, kind="ExternalInput").ap()
    sinT = nc.dram_tensor("sinT", [128, S], BF16, kind="ExternalInput").ap()
    maskT = nc.dram_tensor("maskT", [128, 128], BF16, kind="ExternalInput").ap()
    ones_sq = nc.dram_tensor("ones_sq", [128, 128], BF16, kind="ExternalInput").ap()
    out = nc.dram_tensor("out", [S, D], BF16, kind="ExternalOutput").ap()

    with tile.TileContext(nc) as tc:
        with (
            tc.tile_pool(name="pers", bufs=1) as pers,
            tc.tile_pool(name="work", bufs=1) as wk,
            tc.tile_pool(name="psum", bufs=1, space="PSUM") as psum,
        ):
            # ---- persistent SBUF ----
            cos_sb = pers.tile([128, S], BF16, tag="cos")
            sin_sb = pers.tile([128, S], BF16, tag="sin")
            mask_sb = pers.tile([128, 128], BF16, tag="mask")
            ones_sb = pers.tile([128, 128], BF16, tag="ones")
            kt_sb = pers.tile([128, KVPG, S], BF16, tag="kt")     # K^T roped
            v_sb = pers.tile([128, NKT, 256], BF16, tag="v")      # V natural
            attnT = pers.tile([128, HPG, S], BF16, tag="attnT")

            # ---------------- emission helpers (generators) ----------------
            def interleave(groups):
                """groups: list of (gen, ratio).  Round-robin until all done,
                pulling `ratio` steps of each per round."""
                groups = [(g, r) for g, r in groups if g is not None]
                alive = [True] * len(groups)
                while any(alive):
                    for i, (g, r) in enumerate(groups):
                        if not alive[i]:
                            continue
                        for _ in range(r):
                            try:
                                next(g)
                            except StopIteration:
                                alive[i] = False
                                break

            def proj_head(sb, h, xpans, qt_cur, wh):
                """Project roped head h (0-7 q, 8-9 k) for s-block sb."""
                acc = psum.tile([128, SB], F32, tag="acc", bufs=2, name="acc")
                for dt_i in range(DT):
                    nc.tensor.matmul(
                        acc, wh[:, dt_i, :], xpans[dt_i // 8][:, dt_i % 8, :],
                        start=(dt_i == 0), stop=(dt_i == DT - 1))
                    yield
                # rope: dst = raw*cos + pairswap(raw)*sin_alt, with the
                # rotate-half sign folded into the sin table.  stream_shuffle
                # (DVE partition pair-swap) replaces the PE perm-matmul.
                scols = slice(sb * SB, (sb + 1) * SB)
                raw = wk.tile([128, SB], BF16, tag="raw", bufs=1, name="raw")
                nc.vector.tensor_copy(raw, acc)
                shuf = wk.tile([128, SB], BF16, tag="shuf", bufs=1,
                               name="shuf")
                nc.vector.stream_shuffle(shuf, raw, SHUF_MASK)
                if h < HPG:
                    dst = qt_cur[:, h, :]
                else:
                    dst = kt_sb[:, h - HPG, scols]
                t1 = wk.tile([128, SB], BF16, tag="t1", bufs=1, name="t1")
                nc.vector.tensor_mul(t1, raw, cos_sb[:, scols])
                t2 = wk.tile([128, SB], BF16, tag="t2", bufs=1, name="t2")
                nc.vector.tensor_mul(t2, shuf, sin_sb[:, scols])
                nc.vector.tensor_add(dst, t1, t2)
                yield

            def proj_v(sb, xpans, wv_sb):
                """V natural: per s-tile, V[s,:256] = sum_dt xpanT @ wv."""
                for st in range(SB // 128):
                    vp = psum.tile([128, 256], F32, tag="mm", bufs=2, name="vp")
                    ssl = slice(st * 128, (st + 1) * 128)
                    for dt_i in range(DT):
                        nc.tensor.matmul(
                            vp, xpans[dt_i // 8][:, dt_i % 8, ssl],
                            wv_sb[:, dt_i, :],
                            start=(dt_i == 0), stop=(dt_i == DT - 1))
                        yield
                    st_g = sb * (SB // 128) + st
                    nc.vector.tensor_copy(v_sb[:, st_g, :], vp)
                    yield

            def attn_scores(h, qi, qt_blk, pt):
                """Scores + exp + causal mask for pair (h, qi).  Diagonal
                k-tile m only needs q-columns [m*128, QB) -- the rest is
                fully masked, so skip computing it (triangular trim)."""
                kvs = h // (HPG // KVPG)
                nkt = 4 * qi + 4
                for kti in range(nkt):
                    m = kti - 4 * qi
                    off = max(0, m) * 128
                    sq = psum.tile([128, QB], F32, tag="sq", bufs=2, name="sq")
                    nc.tensor.matmul(
                        sq[:, :QB - off],
                        kt_sb[:, kvs, kti * 128:(kti + 1) * 128],
                        qt_blk[:, h, off:], start=True, stop=True)
                    nc.scalar.activation(
                        pt[:, kti, off:], sq[:, :QB - off],
                        mybir.ActivationFunctionType.Exp, scale=SCALE)
                    if m >= 0:
                        nc.vector.tensor_mul(
                            pt[:, kti, off:off + 128],
                            pt[:, kti, off:off + 128], mask_sb)
                    yield

            def attn_av(h, qi, pt):
                """AV chain + softmax denom + normalize for pair (h, qi)."""
                kvs = h // (HPG // KVPG)
                nkt = 4 * qi + 4
                qcols = slice(qi * QB, (qi + 1) * QB)
                oT = psum.tile([128, QB], F32, tag="oT", bufs=2, name="oT")
                pts = wk.tile([128, QB], BF16, tag="pts", bufs=1, name="pts")
                for kti in range(nkt):
                    m = kti - 4 * qi
                    off = max(0, m) * 128
                    nc.tensor.matmul(
                        oT[:, off:], v_sb[:, kti, kvs * 128:(kvs + 1) * 128],
                        pt[:, kti, off:],
                        start=(kti == 0), stop=(kti == nkt - 1))
                    if kti == 0:
                        nc.vector.tensor_copy(pts, pt[:, 0, :])
                    else:
                        nc.vector.tensor_add(pts[:, off:], pts[:, off:],
                                             pt[:, kti, off:])
                    yield
                bc = psum.tile([128, QB], F32, tag="mm", bufs=2, name="bc")
                nc.tensor.matmul(bc, ones_sb, pts, start=True, stop=True)
                rinv = wk.tile([128, QB], F32, tag="rinv", bufs=1, name="rinv")
                nc.vector.reciprocal_approx_fast(out=rinv, in_=bc)
                nc.vector.tensor_mul(attnT[:, h, qcols], oT, rinv)
                yield

            def st3_block(dblk, qtiles, wo_sb, act_copy=False):
                """o-proj: out[qsl, dblk] = sum_hp attnT[:,hp,qsl].T @ wo.
                act_copy: use the scalar engine for the PSUM->SBUF copy --
                only safe when ACT has no exp work queued (pass 2)."""
                for qt_i in qtiles:
                    qsl = slice(qt_i * 128, (qt_i + 1) * 128)
                    ops = psum.tile([128, 512], F32, tag="acc", bufs=2,
                                    name="ops")
                    for hp in range(HPG):
                        nc.tensor.matmul(
                            ops, attnT[:, hp, qsl], wo_sb[:, hp, :],
                            start=(hp == 0), stop=(hp == HPG - 1))
                        yield
                    o_sb = wk.tile([128, 512], BF16, tag="osb", bufs=2,
                                   name="o_sb")
                    if act_copy:
                        nc.scalar.copy(o_sb, ops)
                    else:
                        nc.vector.tensor_copy(o_sb, ops)
                    nc.sync.dma_start(
                        out=out[qsl, dblk * 512:(dblk + 1) * 512], in_=o_sb)
                    yield

            # ---------------- stage 1+2: projection + attention ----------------
            with tc.tile_pool(name="st1", bufs=1) as st1:
                def load_xpans(sb):
                    # sb0 on the sync queue (immediate); later blocks on the
                    # gpsimd queue, whose FIFO is tile-gated by wh loads --
                    # keeps the startup DMA burst small so the critical
                    # first-chain transfers get the bandwidth.
                    eng = nc.sync if sb == 0 else nc.gpsimd
                    pans = []
                    for hp in range(4):
                        xp = st1.tile([128, 8, SB], BF16, tag="xpan", bufs=6,
                                      name="xp")
                        eng.dma_start(
                            out=xp, in_=x_t[:, sb, hp * 8:(hp + 1) * 8, :])
                        pans.append(xp)
                    return pans

                def load_wh(h, eng=None):
                    wh = st1.tile([128, DT, 128], BF16, tag="wh", bufs=3,
                                  name="wh")
                    (eng or nc.gpsimd).dma_start(out=wh, in_=w_t[:, h])
                    return wh

                # critical-path DMAs first: k0/k1 weights (gpsimd queue) and
                # the sb0 x panels (sync queue); the persistent tables and wv
                # are not needed until ~20us in and must not sit ahead of
                # them in the queue-FIFO at saturated HBM bandwidth.  The k0
                # weights are split in 4 chunks so the first chain can start
                # on chunk 0 while the rest stream.
                # need-ordered critical stream, all on the sync queue:
                # descriptors on one queue complete in issue order at full
                # bandwidth, so interleave k0-weight chunks with the x
                # quarters exactly in first-chain consumption order.
                wh_k0 = st1.tile([128, DT, 128], BF16, tag="wh", bufs=3,
                                 name="wh_k0")
                xpans0 = []
                for ck in range(4):
                    eng_w = nc.sync if ck % 2 == 0 else nc.gpsimd
                    eng_x = nc.gpsimd if ck % 2 == 0 else nc.sync
                    eng_w.dma_start(out=wh_k0[:, ck * 8:(ck + 1) * 8, :],
                                    in_=w_t[:, HPG, ck * 8:(ck + 1) * 8, :])
                    xp = st1.tile([128, 8, SB], BF16, tag="xpan", bufs=6,
                                  name="xp")
                    eng_x.dma_start(out=xp, in_=x_t[:, 0, ck * 8:(ck + 1) * 8, :])
                    xpans0.append(xp)
                xpans = {0: xpans0}
                wh_k1 = load_wh(HPG + 1, eng=nc.sync)
                # PE warm-up: ~60 junk matmuls release the HAM clock gate
                # (4/8 -> 8/8) while the first DMAs land.
                scr = st1.tile([128, 64], BF16, tag="scr", name="scr")
                nc.vector.memset(scr, 0.0)
                wup = psum.tile([128, 64], F32, tag="mm", bufs=2, name="wup")
                for _ in range(60):
                    nc.tensor.matmul(wup[:64, :], scr, scr, start=True,
                                     stop=True, skip_group_check=True)
                nc.gpsimd.dma_start(out=cos_sb, in_=cosT)
                nc.gpsimd.dma_start(out=sin_sb, in_=sinT)
                wh_q0 = load_wh(0, eng=nc.sync)
                nc.gpsimd.dma_start(out=mask_sb, in_=maskT)
                nc.gpsimd.dma_start(out=ones_sb, in_=ones_sq)
                wv_sb = st1.tile([128, DT, 256], BF16, tag="wv")

                def junk_mms(n):
                    for _ in range(n):
                        nc.tensor.matmul(wup[:64, :], scr, scr, start=True,
                                         stop=True, skip_group_check=True)
                        yield

                qt_blocks = {}

                for sb in range(NSB):
                    qt_cur = wk.tile([128, HPG, SB], BF16, tag="qt", bufs=2,
                                     name="qt_cur")
                    qt_blocks[sb] = qt_cur
                    # slots 0..11: [k0] [k1] [v] [q0] .. [q7] [tail] ; attn of
                    # q-block sb-1: scores j in slot 3+j, av j in slot 4+j
                    # slot -> work: 0:k0 1:k1 2:q0..7:q5 8:v 9:q6 10:q7 11:-
                    # (v late so its wv table has ~75us of DMA slack)
                    qslot = lambda j: 2 + j if j < 6 else 3 + j
                    slots = [[] for _ in range(12)]
                    if sb == 0:
                        whk0, whk1 = wh_k0, wh_k1
                    else:
                        whk0, whk1 = load_wh(HPG), load_wh(HPG + 1)
                    slots[0].append((proj_head(sb, HPG, xpans[sb], qt_cur,
                                               whk0), 2))
                    slots[1].append((proj_head(sb, HPG + 1, xpans[sb], qt_cur,
                                               whk1), 2))
                    if sb == 0:
                        # keep HAM warm while the first chains are DMA-paced
                        slots[0].append((junk_mms(16), 1))
                        slots[1].append((junk_mms(8), 1))
                    slots[8].append((proj_v(sb, xpans[sb], wv_sb), 2))
                    for j in range(HPG):
                        whj = wh_q0 if (sb == 0 and j == 0) else load_wh(j)
                        slots[qslot(j)].append(
                            (proj_head(sb, j, xpans[sb], qt_cur, whj), 2))
                        if sb == 0 and j == 1:
                            nc.gpsimd.dma_start(out=wv_sb, in_=wv_t)
                    if sb + 1 < NSB:
                        xpans[sb + 1] = load_xpans(sb + 1)
                    if sb > 0:
                        qi = sb - 1
                        qt_prev = qt_blocks[sb - 1]
                        pt_tiles = {}
                        for j in range(HPG):
                            pt = wk.tile([128, 4 * qi + 4, QB], BF16, tag="pt",
                                         bufs=2, padded_shape=[128, NKT, QB],
                                         name="pt")
                            pt_tiles[j] = pt
                            slots[qslot(j)].append(
                                (attn_scores(j, qi, qt_prev, pt), 1))
                            slots[qslot(j) + 1].append((attn_av(j, qi, pt), 1))
                    for group in slots:
                        interleave(group)

            # ---------------- tail: qi=3 attention + o-proj ----------------
            with tc.tile_pool(name="st3", bufs=1) as st3:
                # wo on the gpsimd queue: the sync FIFO blocks on out-DMA
                # waits, which would delay the next wo issue to block end
                def load_wo(dblk):
                    wo_sb = st3.tile([128, HPG, 512], BF16, tag="wo", bufs=6,
                                     name="wo_sb")
                    nc.gpsimd.dma_start(out=wo_sb, in_=wo_t[:, dblk])
                    return wo_sb

                qi = 3
                qt_prev = qt_blocks[3]
                pt_list = []
                for j in range(HPG):
                    pt = wk.tile([128, NKT, QB], BF16, tag="pt", bufs=2,
                                 name="pt")
                    pt_list.append(pt)
                    group = [(attn_scores(j, qi, qt_prev, pt), 1)]
                    if j > 0:
                        group.append((attn_av(j - 1, qi, pt_list[j - 1]), 1))
                    group.append((st3_block(j, range(12), load_wo(j)), 3))
                    interleave(group)
                # last av interleaved with pass-2 block 0 at ratio 3:1 --
                # av(7) is fully emitted (17 steps, round 6) before block 0
                # reaches its first hp=7 matmul (step 8, round 8), so the
                # in-order PE queue cannot wedge.  Pass 2 uses ACT for the
                # PSUM->SBUF copies: no exp work remains, and the DVE queue
                # (pts adds + fin muls) was gating ops-slot reuse here.
                interleave([(attn_av(HPG - 1, qi, pt_list[HPG - 1]), 3),
                            (st3_block(0, range(12, 16), load_wo(0),
                                       act_copy=True), 1)])
                for dblk in range(1, 8):
                    interleave([(st3_block(dblk, range(12, 16),
                                           load_wo(dblk), act_copy=True), 1)])

    nc.compile()
    return nc


def _host_inputs(x, wq, wk_, wv, wo, cos, sin):
    """Build the 8 per-core input maps (all host-side prep, bf16)."""
    bf = ml_dtypes.bfloat16
    x = np.asarray(x, np.float32)
    wq = np.asarray(wq, np.float32)
    wk_ = np.asarray(wk_, np.float32)
    wv = np.asarray(wv, np.float32)
    wo = np.asarray(wo, np.float32)
    cos = np.asarray(cos, np.float32)
    sin = np.asarray(sin, np.float32)

    cosT = np.repeat(cos.T, 2, axis=0).astype(bf)   # [128, S]
    sinT = np.repeat(sin.T, 2, axis=0)
    sinT[0::2, :] = -sinT[0::2, :]   # rotate-half sign for even lanes
    sinT = sinT.astype(bf)
    kk = np.arange(128)[:, None]
    qq = np.arange(128)[None, :]
    maskT = (qq >= kk).astype(bf)   # within-tile causal triangle
    ones_sq = np.ones((128, 128), bf)

    def tile_w(w_col):  # [D, 128] -> [128, DT, 128]
        return w_col.reshape(DT, 128, 128).transpose(1, 0, 2)

    x_ts = []
    for b in range(B):
        xT = np.ascontiguousarray(x[b].T)                        # [D, S]
        xt = xT.reshape(DT, 128, NSB, SB).transpose(1, 2, 0, 3)  # [128,NSB,DT,SB]
        x_ts.append(np.ascontiguousarray(xt).astype(bf))

    in_maps = []
    for core in range(N_CORES):
        b, g = divmod(core, G)
        w_t = np.empty((128, NQK, DT, 128), np.float32)
        for j in range(HPG):
            w_t[:, j] = tile_w(wq[:, (g * HPG + j) * 128:(g * HPG + j + 1) * 128])
        for j in range(KVPG):
            w_t[:, HPG + j] = tile_w(
                wk_[:, (g * KVPG + j) * 128:(g * KVPG + j + 1) * 128])
        wv_g = wv[:, g * KVPG * 128:(g + 1) * KVPG * 128]        # [D, 256]
        wv_t = wv_g.reshape(DT, 128, 256).transpose(1, 0, 2)     # [128, DT, 256]
        wo_g = wo[g * HPG * HD:(g + 1) * HPG * HD, :]            # [1024, D]
        wo_t = np.ascontiguousarray(
            wo_g.reshape(HPG, 128, 8, 512).transpose(1, 2, 0, 3)
        ).astype(bf)                                             # [128,8,HPG,512]
        in_maps.append({
            "x_t": x_ts[b],
            "w_t": np.ascontiguousarray(w_t).astype(bf),
            "wv_t": np.ascontiguousarray(wv_t).astype(bf),
            "wo_t": wo_t,
            "cosT": cosT, "sinT": sinT, "maskT": maskT,
            "ones_sq": ones_sq,
        })
    return in_maps


def kernel(x, wq, wk, wv, wo, cos, sin, mask, start_pos):
    assert int(start_pos) == 0, "kernel compiled for prefill (start_pos=0)"
    if "nc" not in _CACHE:
        _CACHE["nc"] = _build()
    nc = _CACHE["nc"]
    in_maps = _host_inputs(x, wq, wk, wv, wo, cos, sin)
    res = run_bass_kernel_spmd(nc, in_maps, list(range(N_CORES)))
    outs = [np.asarray(res.results[c]["out"], np.float32)
            for c in range(N_CORES)]
    full = np.empty((B, S, D), np.float32)
    for b in range(B):
        full[b] = outs[4 * b + 0] + outs[4 * b + 1] + outs[4 * b + 2] + outs[4 * b + 3]
    return full
